# revision 20
# baseline (speedup 1.0000x reference)
"""Trainium2 Bass kernel for nn_LowpassDetector.

Computes power = re^2 + im^2 followed by a 4th-order Butterworth lowpass
IIR along the time axis (65536 steps, 512 channels).

Strategy: the IIR poles have max radius 0.7577, so the impulse response
decays below fp32 denormals within 128 taps (sum |h[j]| for j>=128 is
~7e-16).  A 256-tap FIR truncation is therefore numerically exact in
fp32.  The FIR is evaluated as two 128x128 Toeplitz matmuls per
128-timestep chunk (current chunk + previous chunk), which removes the
sequential dependence entirely:

    Y_chunk = H0 @ P_cur + H1 @ P_prev

This lets us shard TIME across the 8 cores (8192 steps each, with a
128-row input halo), giving fully contiguous DMA and zero communication.
Zero-padding the halo of core 0 reproduces the reference's zero initial
state exactly (for t < 256 the truncated FIR equals the IIR identically).

True fp32 matmuls run 4 passes through the PE array; instead each H@P
product is evaluated as a 4-term bf16 split (Hh+Hl) @ (Ph+Pl) with fp32
PSUM accumulation.  bf16 products are exact in the PE (8-bit mantissas),
so the only error is the 2^-18 representation residual: measured
absmax error vs the fp32 reference is 7.4e-6 (rel norm 1.7e-6) instead
of 1.4e-6 for true fp32, at half the PE time.
"""

import numpy as np

T_FULL = 65536
C = 512  # channels
NCORES = 8
TB = T_FULL // NCORES  # 8192 timesteps per core
CH = 128  # chunk length (matmul partition dim)
G = 4  # chunks per DMA group (1 MiB transfers)
GROUP_ROWS = G * CH  # 512
NG = TB // GROUP_ROWS  # 16 groups per core
HALO = CH
IN_ROWS = TB + HALO  # 8320
NTAPS = 2 * CH  # 256

MM_MODE = "bf16x3"  # "bf16x3" | "bf16x4" | "fp32"


def _impulse_response() -> np.ndarray:
    """256-tap impulse response of the reference Butterworth filter (float64)."""
    N, Wn = 4, 0.25
    m = np.arange(-N + 1, N, 2)
    p = -np.exp(1j * np.pi * m / (2 * N))
    fs = 2.0
    warped = 2.0 * fs * np.tan(np.pi * Wn / fs)
    p = p * warped
    k = warped**N
    fs2 = 2.0 * fs
    pz = (fs2 + p) / (fs2 - p)
    zz = -np.ones(N)
    kz = k * (1.0 / np.prod(fs2 - p)).real
    b = kz * np.real(np.poly(zz))
    a = np.real(np.poly(pz))
    b = b / a[0]
    a = a / a[0]
    z = np.zeros(N)
    h = np.zeros(NTAPS)
    for t in range(NTAPS):
        xt = 1.0 if t == 0 else 0.0
        y = b[0] * xt + z[0]
        z = np.concatenate([z[1:], [0.0]]) + b[1:] * xt - a[1:] * y
        h[t] = y
    return h


def _toeplitz() -> tuple[np.ndarray, np.ndarray]:
    """H0, H1 (float64): Y_chunk = H0 @ P_cur + H1 @ P_prev."""
    h = _impulse_response()
    H0 = np.zeros((CH, CH))
    H1 = np.zeros((CH, CH))
    for i in range(CH):
        for ip in range(CH):
            if i - ip >= 0:
                H0[i, ip] = h[i - ip]
            H1[i, ip] = h[i - ip + CH]
    return H0, H1


def _weights_fp32() -> np.ndarray:
    """(2, CH, CH) fp32 lhsT: [H0^T, H1^T]."""
    H0, H1 = _toeplitz()
    return np.stack([H0.T, H1.T]).astype(np.float32)


def _weights_bf16() -> np.ndarray:
    """(4, CH, CH) bf16 lhsT: [Hh0^T, Hl0^T, Hh1^T, Hl1^T]."""
    import ml_dtypes

    bf16 = ml_dtypes.bfloat16
    H0, H1 = _toeplitz()
    out = []
    for H in (H0, H1):
        hi = H.astype(bf16)
        lo = (H - hi.astype(np.float64)).astype(bf16)
        out += [hi.T, lo.T]
    # order: Hh0, Hl0, Hh1, Hl1
    out = [out[0], out[1], out[2], out[3]]
    return np.ascontiguousarray(np.stack(out))


_BUILT = {}


def _build(mode: str = MM_MODE):
    """Build + compile the Bass module (cached per process)."""
    if mode in _BUILT:
        return _BUILT[mode]

    import concourse.bacc as bacc
    import concourse.mybir as mybir
    import concourse.tile as tile

    f32 = mybir.dt.float32
    bf16 = mybir.dt.bfloat16
    split = mode in ("bf16x3", "bf16x4")

    nc = bacc.Bacc(
        "TRN2",
        target_bir_lowering=False,
        debug=False,
        enable_asserts=False,
        num_devices=NCORES,
    )
    sig = nc.dram_tensor("sig", (2, IN_ROWS, C), f32, kind="ExternalInput").ap()
    nwts = 4 if split else 2
    wdt = bf16 if split else f32
    wts = nc.dram_tensor("wts", (nwts, CH, CH), wdt, kind="ExternalInput").ap()
    y = nc.dram_tensor("y", (TB, C), f32, kind="ExternalOutput").ap()

    with tile.TileContext(nc) as tc:
        with (
            tc.tile_pool(name="consts", bufs=1) as cpool,
            tc.tile_pool(name="halo", bufs=1) as halo_pool,
            tc.tile_pool(name="re", bufs=4) as re_pool,
            tc.tile_pool(name="im", bufs=4) as im_pool,
            tc.tile_pool(name="ph", bufs=6) as ph_pool,
            tc.tile_pool(name="pl", bufs=6) as pl_pool,
            tc.tile_pool(name="out", bufs=4) as out_pool,
            tc.tile_pool(name="psum", bufs=8, space="PSUM") as psum_pool,
        ):
            w_t = cpool.tile([CH, nwts, CH], wdt, tag="wts")
            wv = [w_t[:, k, :] for k in range(nwts)]

            # Software-pipelined emission: stage A(g) = load+elementwise,
            # stage B(g) = matmuls, stage C(g) = psum copies + store.
            # Emitting A(g), B(g-1), C(g-2) keeps every engine's FIFO free
            # of waits on downstream stages (emission order sets priority).
            cur_of = {}  # g -> list of per-chunk rhs views
            ps_of = {}  # g -> (out_t, [psum tiles])

            def stage_a(g):
                r0 = HALO + g * GROUP_ROWS  # input row offset
                re_t = re_pool.tile([CH, G, C], f32, tag="re")
                im_t = im_pool.tile([CH, G, C], f32, tag="im")
                nc.sync.dma_start(
                    re_t[:],
                    sig[0, r0 : r0 + GROUP_ROWS, :].rearrange(
                        "(g p) c -> p g c", p=CH
                    ),
                )
                nc.sync.dma_start(
                    im_t[:],
                    sig[1, r0 : r0 + GROUP_ROWS, :].rearrange(
                        "(g p) c -> p g c", p=CH
                    ),
                )
                # power in place: re_t <- re_t^2 + im_t^2
                nc.scalar.square(re_t[:], re_t[:])
                nc.scalar.square(im_t[:], im_t[:])
                nc.vector.tensor_add(re_t[:], re_t[:], im_t[:])
                if split:
                    ph_t = ph_pool.tile([CH, G, C], bf16, tag="ph")
                    pl_t = pl_pool.tile([CH, G, C], bf16, tag="pl")
                    nc.vector.tensor_copy(ph_t[:], re_t[:])
                    nc.vector.tensor_sub(pl_t[:], re_t[:], ph_t[:])
                    cur_of[g] = [(ph_t[:, j, :], pl_t[:, j, :]) for j in range(G)]
                else:
                    cur_of[g] = [(re_t[:, j, :],) for j in range(G)]

            def stage_b(g):
                cur = cur_of[g]
                pss = []
                for j in range(G):
                    ps = psum_pool.tile([CH, C], f32, tag="ps")
                    pv = prev_of[g] if j == 0 else cur[j - 1]
                    if mode == "bf16x4":
                        terms = [
                            (wv[0], cur[j][0]),
                            (wv[0], cur[j][1]),
                            (wv[1], cur[j][0]),
                            (wv[1], cur[j][1]),
                            (wv[2], pv[0]),
                            (wv[2], pv[1]),
                            (wv[3], pv[0]),
                            (wv[3], pv[1]),
                        ]
                    elif mode == "bf16x3":
                        # drop the Hl@pl terms (|Hl@pl| <= 2^-18 |H||p|)
                        terms = [
                            (wv[0], cur[j][0]),
                            (wv[0], cur[j][1]),
                            (wv[1], cur[j][0]),
                            (wv[2], pv[0]),
                            (wv[2], pv[1]),
                            (wv[3], pv[0]),
                        ]
                    else:
                        terms = [(wv[0], cur[j][0]), (wv[1], pv[0])]
                    for k, (w, x) in enumerate(terms):
                        nc.tensor.matmul(
                            ps[:],
                            w,
                            x,
                            start=(k == 0),
                            stop=(k == len(terms) - 1),
                        )
                    pss.append(ps)
                ps_of[g] = pss

            def stage_c(g):
                # stores on the ACT HWDGE ring (keeps the Sync ring pure
                # loads so load issue never stalls on C-stage waits)
                out_t = out_pool.tile([CH, G, C], f32, tag="out")
                for j in range(G):
                    nc.scalar.copy(out_t[:, j, :], ps_of[g][j][:])
                nc.scalar.dma_start(
                    y[g * GROUP_ROWS : (g + 1) * GROUP_ROWS, :].rearrange(
                        "(g p) c -> p g c", p=CH
                    ),
                    out_t[:],
                )
                del ps_of[g]

            def halo_stage():
                # Halo chunk: power of rows [0, 128) = timesteps [-128, 0)
                hre = halo_pool.tile([CH, C], f32, tag="hre")
                him = halo_pool.tile([CH, C], f32, tag="him")
                hp = halo_pool.tile([CH, C], f32, tag="hp")
                nc.sync.dma_start(hre[:], sig[0, 0:CH, :])
                nc.sync.dma_start(him[:], sig[1, 0:CH, :])
                nc.scalar.square(hre[:], hre[:])
                nc.scalar.square(him[:], him[:])
                nc.vector.tensor_add(hp[:], hre[:], him[:])
                if split:
                    hph = halo_pool.tile([CH, C], bf16, tag="hph")
                    hpl = halo_pool.tile([CH, C], bf16, tag="hpl")
                    nc.vector.tensor_copy(hph[:], hp[:])
                    nc.vector.tensor_sub(hpl[:], hp[:], hph[:])
                    return (hph[:], hpl[:])
                return (hp[:],)

            # wts + halo first: tiny transfers, and B(0)'s H1 matmuls need
            # the halo early (PE FIFO would stall on it otherwise)
            nc.sync.dma_start(w_t[:], wts.rearrange("n p m -> p n m"))
            prev_of = {0: halo_stage()}
            for g in range(NG + 2):
                if g < NG:
                    stage_a(g)
                    if g + 1 < NG:
                        prev_of[g + 1] = cur_of[g][G - 1]
                if 1 <= g <= NG:
                    stage_b(g - 1)
                if g >= 2:
                    stage_c(g - 2)

    nc.compile()
    _BUILT[mode] = nc
    return nc


def _prepare_in_maps(signal: np.ndarray, mode: str) -> list[dict[str, np.ndarray]]:
    wts = _weights_bf16() if mode in ("bf16x3", "bf16x4") else _weights_fp32()
    signal = np.ascontiguousarray(np.asarray(signal, dtype=np.float32))
    assert signal.shape == (2, T_FULL, C), signal.shape
    in_maps = []
    for c in range(NCORES):
        t0 = c * TB
        if c == 0:
            block = np.concatenate(
                [np.zeros((2, HALO, C), np.float32), signal[:, 0:TB, :]], axis=1
            )
        else:
            block = signal[:, t0 - HALO : t0 + TB, :]
        in_maps.append({"sig": np.ascontiguousarray(block), "wts": wts})
    return in_maps


def _run(signal: np.ndarray, trace: bool = False, mode: str = MM_MODE):
    """Run the kernel; returns (full_output, BassKernelResults)."""
    from concourse import bass_utils

    nc = _build(mode)
    in_maps = _prepare_in_maps(signal, mode)
    results = bass_utils.run_bass_kernel_spmd(
        nc, in_maps, core_ids=list(range(NCORES)), trace=trace
    )
    y = np.concatenate([r["y"] for r in results.results], axis=0)
    return y, results


def kernel(signal: np.ndarray) -> np.ndarray:
    y, _ = _run(signal, trace=False)
    return y


# revision 36
# speedup vs baseline: 1.0266x; 1.0266x over previous
"""Trainium2 Bass kernel for nn_LowpassDetector.

Computes power = re^2 + im^2 followed by a 4th-order Butterworth lowpass
IIR along the time axis (65536 steps, 512 channels).

Strategy: the IIR poles have max radius 0.7577, so the impulse response
decays below fp32 denormals within 128 taps (sum |h[j]| for j>=128 is
~7e-16).  A 256-tap FIR truncation is therefore numerically exact in
fp32.  The FIR is evaluated as two 128x128 Toeplitz matmuls per
128-timestep chunk (current chunk + previous chunk), which removes the
sequential dependence entirely:

    Y_chunk = H0 @ P_cur + H1 @ P_prev

This lets us shard TIME across the 8 cores (8192 steps each, with a
128-row input halo), giving fully contiguous DMA and zero communication.
Zero-padding the halo of core 0 reproduces the reference's zero initial
state exactly (for t < 256 the truncated FIR equals the IIR identically).

True fp32 matmuls run 4 passes through the PE array; instead each H@P
product is evaluated as a 3-term bf16 split Hh@Ph + Hh@Pl + Hl@Ph with
fp32 PSUM accumulation (H = Hh+Hl, P = Ph+Pl exact to 2^-18).  bf16
products are exact in the PE (8-bit mantissas), so the error is the
2^-18-scale residual terms: measured absmax error vs the fp32 reference
is 1.35e-5 on an output scale of 1.83 (rel norm 2.4e-6), vs 1.4e-6 for
true fp32 matmuls, at 1.5x less PE time (6 vs 4x2 passes per chunk).

The emission is software-pipelined: stage A(g) (loads + power +
hi/lo split), stage B(g-1) (matmuls), stage C(g-2) (PSUM->SBUF copies
+ store) — each engine's in-order queue then never waits on a
downstream stage.  Loads ride the Sync HWDGE ring, stores the ACT ring.
Measured ~152 us/core on 8 trn2 NeuronCores: DMA engines ~134 us busy
moving 51 MB at ~370 GB/s, i.e. at the per-core HBM roofline.
"""

import numpy as np

T_FULL = 65536
C = 512  # channels
NCORES = 8
TB = T_FULL // NCORES  # 8192 timesteps per core
CH = 128  # chunk length (matmul partition dim)
G = 4  # chunks per DMA group (1 MiB transfers)
GROUP_ROWS = G * CH  # 512
NG = TB // GROUP_ROWS  # 16 groups per core
HALO = CH
IN_ROWS = TB + HALO  # 8320
NTAPS = 2 * CH  # 256

MM_MODE = "bf16x3"  # "bf16x3" | "bf16x4" | "fp32"


def _impulse_response() -> np.ndarray:
    """256-tap impulse response of the reference Butterworth filter (float64)."""
    N, Wn = 4, 0.25
    m = np.arange(-N + 1, N, 2)
    p = -np.exp(1j * np.pi * m / (2 * N))
    fs = 2.0
    warped = 2.0 * fs * np.tan(np.pi * Wn / fs)
    p = p * warped
    k = warped**N
    fs2 = 2.0 * fs
    pz = (fs2 + p) / (fs2 - p)
    zz = -np.ones(N)
    kz = k * (1.0 / np.prod(fs2 - p)).real
    b = kz * np.real(np.poly(zz))
    a = np.real(np.poly(pz))
    b = b / a[0]
    a = a / a[0]
    z = np.zeros(N)
    h = np.zeros(NTAPS)
    for t in range(NTAPS):
        xt = 1.0 if t == 0 else 0.0
        y = b[0] * xt + z[0]
        z = np.concatenate([z[1:], [0.0]]) + b[1:] * xt - a[1:] * y
        h[t] = y
    return h


def _toeplitz() -> tuple[np.ndarray, np.ndarray]:
    """H0, H1 (float64): Y_chunk = H0 @ P_cur + H1 @ P_prev."""
    h = _impulse_response()
    H0 = np.zeros((CH, CH))
    H1 = np.zeros((CH, CH))
    for i in range(CH):
        for ip in range(CH):
            if i - ip >= 0:
                H0[i, ip] = h[i - ip]
            H1[i, ip] = h[i - ip + CH]
    return H0, H1


def _weights_fp32() -> np.ndarray:
    """(2, CH, CH) fp32 lhsT: [H0^T, H1^T]."""
    H0, H1 = _toeplitz()
    return np.stack([H0.T, H1.T]).astype(np.float32)


def _weights_bf16() -> np.ndarray:
    """(4, CH, CH) bf16 lhsT: [Hh0^T, Hl0^T, Hh1^T, Hl1^T]."""
    import ml_dtypes

    bf16 = ml_dtypes.bfloat16
    H0, H1 = _toeplitz()
    out = []
    for H in (H0, H1):
        hi = H.astype(bf16)
        lo = (H - hi.astype(np.float64)).astype(bf16)
        out += [hi.T, lo.T]
    # order: Hh0, Hl0, Hh1, Hl1
    out = [out[0], out[1], out[2], out[3]]
    return np.ascontiguousarray(np.stack(out))


_BUILT = {}


def _build(mode: str = MM_MODE):
    """Build + compile the Bass module (cached per process)."""
    if mode in _BUILT:
        return _BUILT[mode]

    import concourse.bacc as bacc
    import concourse.mybir as mybir
    import concourse.tile as tile

    f32 = mybir.dt.float32
    bf16 = mybir.dt.bfloat16
    split = mode in ("bf16x3", "bf16x4")

    nc = bacc.Bacc(
        "TRN2",
        target_bir_lowering=False,
        debug=False,
        enable_asserts=False,
        num_devices=NCORES,
    )
    sig = nc.dram_tensor("sig", (2, IN_ROWS, C), f32, kind="ExternalInput").ap()
    nwts = 4 if split else 2
    wdt = bf16 if split else f32
    wts = nc.dram_tensor("wts", (nwts, CH, CH), wdt, kind="ExternalInput").ap()
    y = nc.dram_tensor("y", (TB, C), f32, kind="ExternalOutput").ap()

    with tile.TileContext(nc) as tc:
        with (
            tc.tile_pool(name="consts", bufs=1) as cpool,
            tc.tile_pool(name="halo", bufs=1) as halo_pool,
            tc.tile_pool(name="re", bufs=4) as re_pool,
            tc.tile_pool(name="im", bufs=4) as im_pool,
            tc.tile_pool(name="ph", bufs=8) as ph_pool,
            tc.tile_pool(name="pl", bufs=8) as pl_pool,
            tc.tile_pool(name="out", bufs=4) as out_pool,
            tc.tile_pool(name="psum", bufs=8, space="PSUM") as psum_pool,
        ):
            w_t = cpool.tile([CH, nwts, CH], wdt, tag="wts")
            wv = [w_t[:, k, :] for k in range(nwts)]

            # Software-pipelined emission: stage A(g) = load+elementwise,
            # stage B(g) = matmuls, stage C(g) = psum copies + store.
            # Emitting A(g), B(g-1), C(g-2) keeps every engine's FIFO free
            # of waits on downstream stages (emission order sets priority).
            cur_of = {}  # g -> list of per-chunk rhs views
            ps_of = {}  # g -> (out_t, [psum tiles])

            def stage_a(g):
                r0 = HALO + g * GROUP_ROWS  # input row offset
                re_t = re_pool.tile([CH, G, C], f32, tag="re")
                im_t = im_pool.tile([CH, G, C], f32, tag="im")
                src = sig[:, r0 : r0 + GROUP_ROWS, :].rearrange(
                    "s (g p) c -> s p g c", p=CH
                )
                nc.sync.dma_start(re_t[:], src[0])
                nc.sync.dma_start(im_t[:], src[1])
                # power in place: re_t <- re_t^2 + im_t^2
                nc.scalar.square(re_t[:], re_t[:])
                nc.scalar.square(im_t[:], im_t[:])
                nc.vector.tensor_add(re_t[:], re_t[:], im_t[:])
                if split:
                    ph_t = ph_pool.tile([CH, G, C], bf16, tag="ph")
                    pl_t = pl_pool.tile([CH, G, C], bf16, tag="pl")
                    nc.vector.tensor_copy(ph_t[:], re_t[:])
                    nc.vector.tensor_sub(pl_t[:], re_t[:], ph_t[:])
                    cur_of[g] = [(ph_t[:, j, :], pl_t[:, j, :]) for j in range(G)]
                else:
                    cur_of[g] = [(re_t[:, j, :],) for j in range(G)]

            def stage_b(g):
                cur = cur_of[g]
                pss = []
                for j in range(G):
                    ps = psum_pool.tile([CH, C], f32, tag="ps")
                    pv = prev_of[g] if j == 0 else cur[j - 1]
                    if mode == "bf16x4":
                        terms = [
                            (wv[0], cur[j][0]),
                            (wv[0], cur[j][1]),
                            (wv[1], cur[j][0]),
                            (wv[1], cur[j][1]),
                            (wv[2], pv[0]),
                            (wv[2], pv[1]),
                            (wv[3], pv[0]),
                            (wv[3], pv[1]),
                        ]
                    elif mode == "bf16x3":
                        # drop the Hl@pl terms (|Hl@pl| <= 2^-18 |H||p|)
                        terms = [
                            (wv[0], cur[j][0]),
                            (wv[0], cur[j][1]),
                            (wv[1], cur[j][0]),
                            (wv[2], pv[0]),
                            (wv[2], pv[1]),
                            (wv[3], pv[0]),
                        ]
                    else:
                        terms = [(wv[0], cur[j][0]), (wv[1], pv[0])]
                    for k, (w, x) in enumerate(terms):
                        nc.tensor.matmul(
                            ps[:],
                            w,
                            x,
                            start=(k == 0),
                            stop=(k == len(terms) - 1),
                        )
                    pss.append(ps)
                ps_of[g] = pss

            def stage_c(g):
                # stores on the ACT HWDGE ring (keeps the Sync ring pure
                # loads so load issue never stalls on C-stage waits)
                out_t = out_pool.tile([CH, G, C], f32, tag="out")
                for j in range(G):
                    nc.scalar.copy(out_t[:, j, :], ps_of[g][j][:])
                nc.scalar.dma_start(
                    y[g * GROUP_ROWS : (g + 1) * GROUP_ROWS, :].rearrange(
                        "(g p) c -> p g c", p=CH
                    ),
                    out_t[:],
                )
                del ps_of[g]

            def halo_stage():
                # Halo chunk: power of rows [0, 128) = timesteps [-128, 0)
                hre = halo_pool.tile([CH, C], f32, tag="hre")
                him = halo_pool.tile([CH, C], f32, tag="him")
                hp = halo_pool.tile([CH, C], f32, tag="hp")
                nc.sync.dma_start(hre[:], sig[0, 0:CH, :])
                nc.sync.dma_start(him[:], sig[1, 0:CH, :])
                nc.scalar.square(hre[:], hre[:])
                nc.scalar.square(him[:], him[:])
                nc.vector.tensor_add(hp[:], hre[:], him[:])
                if split:
                    hph = halo_pool.tile([CH, C], bf16, tag="hph")
                    hpl = halo_pool.tile([CH, C], bf16, tag="hpl")
                    nc.vector.tensor_copy(hph[:], hp[:])
                    nc.vector.tensor_sub(hpl[:], hp[:], hph[:])
                    return (hph[:], hpl[:])
                return (hp[:],)

            # wts + halo first: tiny transfers, and B(0)'s H1 matmuls need
            # the halo early (PE FIFO would stall on it otherwise)
            nc.sync.dma_start(w_t[:], wts.rearrange("n p m -> p n m"))
            prev_of = {0: halo_stage()}
            for g in range(NG + 2):
                if g < NG:
                    stage_a(g)
                    if g + 1 < NG:
                        prev_of[g + 1] = cur_of[g][G - 1]
                if 1 <= g <= NG:
                    stage_b(g - 1)
                if g >= 2:
                    stage_c(g - 2)

    nc.compile()
    _BUILT[mode] = nc
    return nc


def _prepare_in_maps(signal: np.ndarray, mode: str) -> list[dict[str, np.ndarray]]:
    wts = _weights_bf16() if mode in ("bf16x3", "bf16x4") else _weights_fp32()
    signal = np.ascontiguousarray(np.asarray(signal, dtype=np.float32))
    assert signal.shape == (2, T_FULL, C), signal.shape
    in_maps = []
    for c in range(NCORES):
        t0 = c * TB
        if c == 0:
            block = np.concatenate(
                [np.zeros((2, HALO, C), np.float32), signal[:, 0:TB, :]], axis=1
            )
        else:
            block = signal[:, t0 - HALO : t0 + TB, :]
        in_maps.append({"sig": np.ascontiguousarray(block), "wts": wts})
    return in_maps


def _run(signal: np.ndarray, trace: bool = False, mode: str | None = None):
    """Run the kernel; returns (full_output, BassKernelResults)."""
    from concourse import bass_utils

    if mode is None:
        mode = MM_MODE
    nc = _build(mode)
    in_maps = _prepare_in_maps(signal, mode)
    results = bass_utils.run_bass_kernel_spmd(
        nc, in_maps, core_ids=list(range(NCORES)), trace=trace
    )
    y = np.concatenate([r["y"] for r in results.results], axis=0)
    return y, results


def kernel(signal: np.ndarray) -> np.ndarray:
    y, _ = _run(signal, trace=False)
    return y


# revision 38
# speedup vs baseline: 1.1509x; 1.1211x over previous
"""Trainium2 Bass kernel for nn_LowpassDetector.

Computes power = re^2 + im^2 followed by a 4th-order Butterworth lowpass
IIR along the time axis (65536 steps, 512 channels).

Strategy: the IIR poles have max radius 0.7577, so the impulse response
decays below fp32 denormals within 128 taps (sum |h[j]| for j>=128 is
~7e-16).  A 256-tap FIR truncation is therefore numerically exact in
fp32.  The FIR is evaluated as two 128x128 Toeplitz matmuls per
128-timestep chunk (current chunk + previous chunk), which removes the
sequential dependence entirely:

    Y_chunk = H0 @ P_cur + H1 @ P_prev

This lets us shard TIME across the 8 cores (8192 steps each, with a
128-row input halo), giving fully contiguous DMA and zero communication.
Zero-padding the halo of core 0 reproduces the reference's zero initial
state exactly (for t < 256 the truncated FIR equals the IIR identically).

True fp32 matmuls run 4 passes through the PE array; instead each H@P
product is evaluated as a 3-term bf16 split Hh@Ph + Hh@Pl + Hl@Ph with
fp32 PSUM accumulation (H = Hh+Hl, P = Ph+Pl exact to 2^-18).  bf16
products are exact in the PE (8-bit mantissas), so the error is the
2^-18-scale residual terms: measured absmax error vs the fp32 reference
is 1.35e-5 on an output scale of 1.83 (rel norm 2.4e-6), vs 1.4e-6 for
true fp32 matmuls, at 1.5x less PE time (6 vs 4x2 passes per chunk).

The emission is software-pipelined: stage A(g) (loads + power +
hi/lo split), stage B(g-1) (matmuls), stage C(g-2) (PSUM->SBUF copies
+ store) — each engine's in-order queue then never waits on a
downstream stage.  Loads ride the Sync HWDGE ring, stores the ACT ring.
Measured ~152 us/core on 8 trn2 NeuronCores: DMA engines ~134 us busy
moving 51 MB at ~370 GB/s, i.e. at the per-core HBM roofline.
"""

import numpy as np

T_FULL = 65536
C = 512  # channels
NCORES = 8
TB = T_FULL // NCORES  # 8192 timesteps per core
CH = 128  # chunk length (matmul partition dim)
G = 4  # chunks per DMA group (1 MiB transfers)
GROUP_ROWS = G * CH  # 512
NG = TB // GROUP_ROWS  # 16 groups per core
HALO = CH
IN_ROWS = TB + HALO  # 8320
NTAPS = 2 * CH  # 256

MM_MODE = "bf16x3"  # "bf16x3" | "bf16x4" | "fp32"


def _impulse_response() -> np.ndarray:
    """256-tap impulse response of the reference Butterworth filter (float64)."""
    N, Wn = 4, 0.25
    m = np.arange(-N + 1, N, 2)
    p = -np.exp(1j * np.pi * m / (2 * N))
    fs = 2.0
    warped = 2.0 * fs * np.tan(np.pi * Wn / fs)
    p = p * warped
    k = warped**N
    fs2 = 2.0 * fs
    pz = (fs2 + p) / (fs2 - p)
    zz = -np.ones(N)
    kz = k * (1.0 / np.prod(fs2 - p)).real
    b = kz * np.real(np.poly(zz))
    a = np.real(np.poly(pz))
    b = b / a[0]
    a = a / a[0]
    z = np.zeros(N)
    h = np.zeros(NTAPS)
    for t in range(NTAPS):
        xt = 1.0 if t == 0 else 0.0
        y = b[0] * xt + z[0]
        z = np.concatenate([z[1:], [0.0]]) + b[1:] * xt - a[1:] * y
        h[t] = y
    return h


def _toeplitz() -> tuple[np.ndarray, np.ndarray]:
    """H0, H1 (float64): Y_chunk = H0 @ P_cur + H1 @ P_prev."""
    h = _impulse_response()
    H0 = np.zeros((CH, CH))
    H1 = np.zeros((CH, CH))
    for i in range(CH):
        for ip in range(CH):
            if i - ip >= 0:
                H0[i, ip] = h[i - ip]
            H1[i, ip] = h[i - ip + CH]
    return H0, H1


def _weights_fp32() -> np.ndarray:
    """(2, CH, CH) fp32 lhsT: [H0^T, H1^T]."""
    H0, H1 = _toeplitz()
    return np.stack([H0.T, H1.T]).astype(np.float32)


def _weights_bf16() -> np.ndarray:
    """(4, CH, CH) bf16 lhsT: [Hh0^T, Hl0^T, Hh1^T, Hl1^T]."""
    import ml_dtypes

    bf16 = ml_dtypes.bfloat16
    H0, H1 = _toeplitz()
    out = []
    for H in (H0, H1):
        hi = H.astype(bf16)
        lo = (H - hi.astype(np.float64)).astype(bf16)
        out += [hi.T, lo.T]
    # order: Hh0, Hl0, Hh1, Hl1
    out = [out[0], out[1], out[2], out[3]]
    return np.ascontiguousarray(np.stack(out))


_BUILT = {}


def _build(mode: str = MM_MODE):
    """Build + compile the Bass module (cached per process)."""
    if mode in _BUILT:
        return _BUILT[mode]

    import concourse.bacc as bacc
    import concourse.mybir as mybir
    import concourse.tile as tile

    f32 = mybir.dt.float32
    bf16 = mybir.dt.bfloat16
    split = mode in ("bf16x3", "bf16x4")

    nc = bacc.Bacc(
        "TRN2",
        target_bir_lowering=False,
        debug=False,
        enable_asserts=False,
        num_devices=NCORES,
    )
    sig = nc.dram_tensor("sig", (2, IN_ROWS, C), f32, kind="ExternalInput").ap()
    nwts = 4 if split else 2
    wdt = bf16 if split else f32
    wts = nc.dram_tensor("wts", (nwts, CH, CH), wdt, kind="ExternalInput").ap()
    y = nc.dram_tensor("y", (TB, C), f32, kind="ExternalOutput").ap()

    with tile.TileContext(nc) as tc:
        with (
            tc.tile_pool(name="consts", bufs=1) as cpool,
            tc.tile_pool(name="halo", bufs=1) as halo_pool,
            tc.tile_pool(name="re", bufs=4) as re_pool,
            tc.tile_pool(name="im", bufs=4) as im_pool,
            tc.tile_pool(name="ph", bufs=8) as ph_pool,
            tc.tile_pool(name="pl", bufs=8) as pl_pool,
            tc.tile_pool(name="out", bufs=4) as out_pool,
            tc.tile_pool(name="psum", bufs=8, space="PSUM") as psum_pool,
        ):
            w_t = cpool.tile([CH, nwts, CH], wdt, tag="wts")
            wv = [w_t[:, k, :] for k in range(nwts)]

            # Software-pipelined emission: stage A(g) = load+elementwise,
            # stage B(g) = matmuls, stage C(g) = psum copies + store.
            # Emitting A(g), B(g-1), C(g-2) keeps every engine's FIFO free
            # of waits on downstream stages (emission order sets priority).
            cur_of = {}  # g -> list of per-chunk rhs views
            ps_of = {}  # g -> (out_t, [psum tiles])

            def stage_a(g):
                r0 = HALO + g * GROUP_ROWS  # input row offset
                re_t = re_pool.tile([CH, G, C], f32, tag="re")
                im_t = im_pool.tile([CH, G, C], f32, tag="im")
                src = sig[:, r0 : r0 + GROUP_ROWS, :].rearrange(
                    "s (g p) c -> s p g c", p=CH
                )
                nc.sync.dma_start(re_t[:], src[0])
                nc.sync.dma_start(im_t[:], src[1])
                # power in place: re_t <- re_t^2 + im_t^2
                nc.scalar.square(re_t[:], re_t[:])
                nc.scalar.square(im_t[:], im_t[:])
                nc.vector.tensor_add(re_t[:], re_t[:], im_t[:])
                if split:
                    ph_t = ph_pool.tile([CH, G, C], bf16, tag="ph")
                    pl_t = pl_pool.tile([CH, G, C], bf16, tag="pl")
                    nc.vector.tensor_copy(ph_t[:], re_t[:])
                    nc.vector.tensor_sub(pl_t[:], re_t[:], ph_t[:])
                    cur_of[g] = [(ph_t[:, j, :], pl_t[:, j, :]) for j in range(G)]
                else:
                    cur_of[g] = [(re_t[:, j, :],) for j in range(G)]

            def stage_b(g):
                cur = cur_of[g]
                pss = []
                for j in range(G):
                    ps = psum_pool.tile([CH, C], f32, tag="ps")
                    pv = prev_of[g] if j == 0 else cur[j - 1]
                    if mode == "bf16x4":
                        terms = [
                            (wv[0], cur[j][0]),
                            (wv[0], cur[j][1]),
                            (wv[1], cur[j][0]),
                            (wv[1], cur[j][1]),
                            (wv[2], pv[0]),
                            (wv[2], pv[1]),
                            (wv[3], pv[0]),
                            (wv[3], pv[1]),
                        ]
                    elif mode == "bf16x3":
                        # drop the Hl@pl terms (|Hl@pl| <= 2^-18 |H||p|)
                        terms = [
                            (wv[0], cur[j][0]),
                            (wv[0], cur[j][1]),
                            (wv[1], cur[j][0]),
                            (wv[2], pv[0]),
                            (wv[2], pv[1]),
                            (wv[3], pv[0]),
                        ]
                    else:
                        terms = [(wv[0], cur[j][0]), (wv[1], pv[0])]
                    for k, (w, x) in enumerate(terms):
                        nc.tensor.matmul(
                            ps[:],
                            w,
                            x,
                            start=(k == 0),
                            stop=(k == len(terms) - 1),
                        )
                    pss.append(ps)
                ps_of[g] = pss

            def stage_c(g):
                # stores on the ACT HWDGE ring (keeps the Sync ring pure
                # loads so load issue never stalls on C-stage waits)
                out_t = out_pool.tile([CH, G, C], f32, tag="out")
                for j in range(G):
                    nc.scalar.copy(out_t[:, j, :], ps_of[g][j][:])
                nc.scalar.dma_start(
                    y[g * GROUP_ROWS : (g + 1) * GROUP_ROWS, :].rearrange(
                        "(g p) c -> p g c", p=CH
                    ),
                    out_t[:],
                )
                del ps_of[g]

            def halo_stage():
                # Halo chunk: power of rows [0, 128) = timesteps [-128, 0)
                hre = halo_pool.tile([CH, C], f32, tag="hre")
                him = halo_pool.tile([CH, C], f32, tag="him")
                hp = halo_pool.tile([CH, C], f32, tag="hp")
                nc.sync.dma_start(hre[:], sig[0, 0:CH, :])
                nc.sync.dma_start(him[:], sig[1, 0:CH, :])
                nc.scalar.square(hre[:], hre[:])
                nc.scalar.square(him[:], him[:])
                nc.vector.tensor_add(hp[:], hre[:], him[:])
                if split:
                    hph = halo_pool.tile([CH, C], bf16, tag="hph")
                    hpl = halo_pool.tile([CH, C], bf16, tag="hpl")
                    nc.vector.tensor_copy(hph[:], hp[:])
                    nc.vector.tensor_sub(hpl[:], hp[:], hph[:])
                    return (hph[:], hpl[:])
                return (hp[:],)

            # wts + halo first: tiny transfers, and B(0)'s H1 matmuls need
            # the halo early (PE FIFO would stall on it otherwise)
            nc.sync.dma_start(w_t[:], wts.rearrange("n p m -> p n m"))
            prev_of = {0: halo_stage()}
            for g in range(NG + 2):
                if g < NG:
                    stage_a(g)
                    if g + 1 < NG:
                        prev_of[g + 1] = cur_of[g][G - 1]
                if 1 <= g <= NG:
                    stage_b(g - 1)
                if g >= 2:
                    stage_c(g - 2)

    nc.compile()
    _BUILT[mode] = nc
    return nc


def _prepare_in_maps(signal: np.ndarray, mode: str) -> list[dict[str, np.ndarray]]:
    wts = _weights_bf16() if mode in ("bf16x3", "bf16x4") else _weights_fp32()
    signal = np.ascontiguousarray(np.asarray(signal, dtype=np.float32))
    assert signal.shape == (2, T_FULL, C), signal.shape
    in_maps = []
    for c in range(NCORES):
        t0 = c * TB
        if c == 0:
            block = np.concatenate(
                [np.zeros((2, HALO, C), np.float32), signal[:, 0:TB, :]], axis=1
            )
        else:
            block = signal[:, t0 - HALO : t0 + TB, :]
        in_maps.append({"sig": np.ascontiguousarray(block), "wts": wts})
    return in_maps


def _run(signal: np.ndarray, trace: bool = False, mode: str | None = None):
    """Run the kernel; returns (full_output, BassKernelResults)."""
    from concourse import bass_utils

    if mode is None:
        mode = MM_MODE
    nc = _build(mode)
    in_maps = _prepare_in_maps(signal, mode)
    results = bass_utils.run_bass_kernel_spmd(
        nc, in_maps, core_ids=list(range(NCORES)), trace=trace
    )
    y = np.concatenate([r["y"] for r in results.results], axis=0)
    return y, results


def kernel(signal: np.ndarray) -> np.ndarray:
    y, _ = _run(signal, trace=False)
    return y
